# revision 1
# baseline (speedup 1.0000x reference)
"""Trainium2 Bass kernel for EpiLinear (epinet + prior-ensemble MLP).

Strategy (per spec sharding hint: data-parallel over batch, params replicated):
  - Shard B=2048 across 8 cores (256 rows each).
  - Key algebraic restructure: the epinet input is concat([xf, z]) where
    xf = concat(x, feature) is broadcast over the n=8 z-samples. So
      h = relu(epinet_inp @ Wep1 + b) = relu(A[b] + Bz[b,n] + b)
    with A = xf @ Wep1[:2048] computed ONCE per batch row (8x fewer FLOPs
    than the naive [B*n, 2080] GEMM) and Bz = z @ Wep1[2048:] tiny.
  - All activations are kept feature-on-partitions (transposed) so every
    GEMM contraction maps to the PE partition axis with no on-chip
    transposes; the host pre-transposes x/feature/z (cheap numpy prep).
  - Prior ensemble (32 tiny MLPs 1024->5->5->1) is flattened into dense
    GEMMs via host-built block-diagonal weight matrices.
  - Final reduction over the 32 noise dims is a partition-group sum done
    with one matmul against a group-selection matrix.
  - Heavy tensors (w1, xfT, wp1, zT, w2, hidden acts) travel/compute in
    bf16 (fp32 PSUM accumulation); small/sensitive paths use float32r.
  - Bz matmuls (K=32) are row-packed 4x and out2 matmuls (M=32) are
    col-packed 4x via tile_position so they run concurrently in the PE.
  - Per hid-tile software pipeline: PE does [Bz(m), A(m)] while DVE/GPSIMD
    run h(m-1) = relu(A + b + Bz), so elementwise work hides under GEMMs.
  - Small parameters ride in one packed DMA; w1/xfT are host-swizzled
    into SBUF layout so every bulk DMA is fully contiguous on both sides.
"""

import time

import numpy as np
import ml_dtypes

import concourse.bacc as bacc
import concourse.mybir as mybir
import concourse.tile as tile
from concourse.bass_utils import run_bass_kernel_spmd

F32 = mybir.dt.float32
F32R = mybir.dt.float32r
BF16 = mybir.dt.bfloat16
RELU = mybir.ActivationFunctionType.Relu
COPY = mybir.ActivationFunctionType.Copy
ADD = mybir.AluOpType.add
MULT = mybir.AluOpType.mult

USE_BF16 = True
DT = BF16 if USE_BF16 else F32R
NPDT = ml_dtypes.bfloat16 if USE_BF16 else np.float32

N_CORES = 8
B, N_Z, ND, SD, HD = 2048, 8, 32, 1024, 1024
EH = 512                  # epinet hidden
XF = SD + HD              # 2048 concat(x, feature) features
BL = B // N_CORES         # 256 batch rows per core
R = BL * N_Z              # 2048 epinet rows per core (r = n*BL + b, n-major)
PHF = 160                 # 32 ensembles * 5 prior hidden, flattened
KT = XF // 128            # 16 k-tiles over xf features
MT = EH // 128            # 4 hid tiles of epinet hidden
RC = R // 512             # 4 chunks of 512 epinet rows

# packed bf16 param block column offsets: w1z4 | w2 | wp2a | wp2b | wp3a |
# wp3b | wp1
PB_W1Z, PB_W2, PB_W2A, PB_W2B = 0, 512, 640, 800
PB_W3A, PB_W3B, PB_WP1, PB_COLS = 960, 992, 1024, 2304

_CACHE = {}


def _build():
    nc = bacc.Bacc("TRN2", target_bir_lowering=False, debug=False,
                   num_devices=N_CORES)
    f = lambda name, shape, dt: nc.dram_tensor(name, shape, dt, kind="ExternalInput").ap()
    xfT = f("xfT", [128, KT * BL], DT)  # xf.T slice, SBUF-layout swizzled
    w1 = f("w1", [128, MT, KT * 128], DT)  # Wep1[:2048] SBUF-layout swizzled
    zT4 = f("zT4", [128, R], DT)        # z^T (r n-major) replicated 4x
    zp = f("zp", [128, 512], F32)       # z^T packed into 4 partition strips
    packb = f("packb", [128, PB_COLS], DT)   # all small bf16 params
    bias = f("bias", [128, 9], F32)     # packed per-partition biases
    sel4 = f("sel4", [128, 4], F32R)    # group-sum selection matrix
    out = nc.dram_tensor("out", [RC, 512], F32, kind="ExternalOutput").ap()

    with tile.TileContext(nc) as tc:
        with (
            tc.tile_pool(name="const", bufs=1) as cp,
            tc.tile_pool(name="work", bufs=1) as wk,
            tc.tile_pool(name="tmp", bufs=4) as tp,
            tc.tile_pool(name="ps_a", bufs=2, space="PSUM") as ps_a,
            tc.tile_pool(name="ps_bz", bufs=5, space="PSUM") as ps_bz,
            tc.tile_pool(name="ps_o2", bufs=1, space="PSUM") as ps_o2,
        ):
            # ---- SBUF tiles -------------------------------------------------
            xfT_sb = cp.tile([128, KT * BL], DT)      # [p, (k b)]
            w1_sb = cp.tile([128, MT * KT * 128], DT)  # [p, (m k h)]
            zT4_sb = cp.tile([128, R], DT)
            pk_sb = cp.tile([128, PB_COLS], DT)
            zp_sb = cp.tile([128, 512], F32)
            bias_sb = cp.tile([128, 9], F32)
            sel4_sb = cp.tile([128, 4], F32R)

            A_sb = wk.tile([128, MT * BL], F32)       # A^T (no bias), [p, (m b)]
            h_sb = [wk.tile([128, R], DT, name=f"h{m}") for m in range(MT)]
            h1a_sb = wk.tile([128, BL], DT)
            h1b_sb = wk.tile([32, BL], DT)
            h2a_sb = wk.tile([128, BL], DT)
            h2b_sb = wk.tile([32, BL], DT)
            prep_sb = wk.tile([128, 512], F32)        # prior out, strip+col replicated
            g_sb = wk.tile([128, 512], F32)
            gm_sb = wk.tile([128, 512], F32R)
            out_sb = wk.tile([RC, 512], F32)

            x3 = xfT_sb[:].rearrange("p (k b) -> p k b", b=BL)
            w13 = w1_sb[:].rearrange("p (m k h) -> p m k h", m=MT, h=128)
            w1z4_v = pk_sb[:, PB_W1Z:PB_W1Z + EH]
            w23 = pk_sb[:, PB_W2:PB_W2A].rearrange("p (k o) -> p k o", o=ND)
            wp2a_v = pk_sb[:, PB_W2A:PB_W2B]
            wp2b_v = pk_sb[0:32, PB_W2B:PB_W3A]
            wp3a_v = pk_sb[:, PB_W3A:PB_W3B]
            wp3b_v = pk_sb[0:32, PB_W3B:PB_WP1]
            wp13 = pk_sb[:, PB_WP1:PB_COLS].rearrange("p (k g) -> p k g", g=PHF)

            # ---- DMAs: small/early on scalar queue, bulk on sync queue -----
            nc.scalar.dma_start(pk_sb[:, 0:PB_W2A], packb[:, 0:PB_W2A])
            nc.scalar.dma_start(zT4_sb[:], zT4[:])
            nc.scalar.dma_start(pk_sb[:, PB_W2A:], packb[:, PB_W2A:])
            nc.gpsimd.dma_start(bias_sb[:], bias[:])
            nc.gpsimd.dma_start(sel4_sb[:], sel4[:])
            nc.gpsimd.dma_start(zp_sb[:], zp[:])

            HKT = KT * 128 // 2
            def _xf(c):
                nc.sync.dma_start(
                    xfT_sb[:, 4 * c * BL:(4 * c + 4) * BL],
                    xfT[:, 4 * c * BL:(4 * c + 4) * BL])
            def _w1(c, half):
                nc.sync.dma_start(
                    w1_sb[:, (c * 2 + half) * HKT:(c * 2 + half + 1) * HKT],
                    w1[:, c, half * HKT:(half + 1) * HKT])
            # w1[m0] early (PE can start), then the rest of xf (so A(m0)
            # COMPLETES early and the h pipeline drains), then w1 m1..m3
            # half-chunks which pace the remaining stages.
            _xf(0); _w1(0, 0); _w1(0, 1); _xf(1); _xf(2); _xf(3)
            for c in range(1, 4):
                _w1(c, 0); _w1(c, 1)

            # ---- epinet L1, software-pipelined per hid-tile m:
            #   PE: [Bz(m) -> A(m)]  ||  DVE/GPSIMD: h(m-1) = relu(A+b+Bz)
            # Bz(m) runs while the w1[m] DMA chunk is still in flight.
            for m in range(MT):
                psz = [ps_bz.tile([128, 512], F32, tag="pz", name=f"pz{m}_{rc}")
                       for rc in range(RC)]
                for rc in range(RC):
                    nc.tensor.matmul(
                        psz[rc][:, :],
                        w1z4_v[32 * rc:32 * rc + 32, 128 * m:128 * m + 128],
                        zT4_sb[32 * rc:32 * rc + 32, 512 * rc:512 * rc + 512],
                        start=True, stop=True, tile_position=(32 * rc, 0))
                psA = ps_a.tile([128, BL], F32, tag="pa", name=f"pA{m}")
                for k in range(KT):
                    nc.tensor.matmul(
                        psA[:, :], w13[:, m, k, :],
                        x3[:, k, :], start=(k == 0), stop=(k == KT - 1))
                nc.scalar.activation(A_sb[:, BL * m:BL * (m + 1)], psA[:, :],
                                     COPY)
                Ab = A_sb[:, BL * m:BL * (m + 1)].unsqueeze(1).broadcast_to(
                    (128, 2, BL))
                for rc in range(RC):
                    t = tp.tile([128, 512], DT, tag="t")
                    nc.vector.scalar_tensor_tensor(
                        t[:].rearrange("p (a b) -> p a b", a=2),
                        psz[rc][:, :].rearrange("p (a b) -> p a b", a=2),
                        bias_sb[:, m:m + 1], Ab, op0=ADD, op1=ADD)
                    nc.gpsimd.tensor_scalar_max(
                        h_sb[m][:, 512 * rc:512 * rc + 512], t[:], 0.0)

            # ---- prior ensemble: h1 = relu(x @ wp1 + b) --------------------
            ps1 = []
            for m, (mp, m0) in enumerate([(128, 0), (32, 128)]):
                ps = ps_a.tile([128, BL], F32, tag="pa", name=f"pp1_{m}")
                for k in range(8):  # x = first 1024 features of xfT
                    nc.tensor.matmul(
                        ps[0:mp, :], wp13[:, k, m0:m0 + mp], x3[:, k, :],
                        start=(k == 0), stop=(k == 7))
                ps1.append(ps)
            nc.scalar.activation(h1a_sb[:], ps1[0][0:128, :], RELU,
                                 bias=bias_sb[:, 4:5])
            nc.scalar.activation(h1b_sb[:], ps1[1][0:32, :], RELU,
                                 bias=bias_sb[0:32, 5:6])

            # h2 = relu(h1 @ wp2 + b)   (block-diag dense)
            ps2 = []
            for m, (mp, m0) in enumerate([(128, 0), (32, 128)]):
                ps = ps_a.tile([128, BL], F32, tag="pa", name=f"pp2_{m}")
                nc.tensor.matmul(ps[0:mp, :], wp2a_v[:, m0:m0 + mp],
                                 h1a_sb[:], start=True, stop=False)
                nc.tensor.matmul(ps[0:mp, :], wp2b_v[:, m0:m0 + mp],
                                 h1b_sb[:], start=False, stop=True)
                ps2.append(ps)
            nc.scalar.activation(h2a_sb[:], ps2[0][0:128, :], RELU,
                                 bias=bias_sb[:, 6:7])
            nc.scalar.activation(h2b_sb[:], ps2[1][0:32, :], RELU,
                                 bias=bias_sb[0:32, 7:8])

            # p = h2 @ wp3   -> [32 ensembles, BL]
            psp = ps_a.tile([128, BL], F32, tag="pa", name="ppp")
            nc.tensor.matmul(psp[0:32, :], wp3a_v[:], h2a_sb[:],
                             start=True, stop=False)
            nc.tensor.matmul(psp[0:32, :], wp3b_v[:], h2b_sb[:],
                             start=False, stop=True)
            # p replicated into 4 partition strips x 2 column copies
            pb = psp[0:32, :].unsqueeze(1).broadcast_to((32, 2, BL))
            for c in range(RC):
                nc.scalar.activation(
                    prep_sb[32 * c:32 * c + 32, :].rearrange(
                        "p (a b) -> p a b", a=2), pb, COPY)

            # ---- epinet L2 col-packed 4x: out2^T[32rc+k, q] ----------------
            pso = ps_o2.tile([128, 512], F32, tag="po", name="po")
            for k in range(MT):
                for rc in range(RC):
                    nc.tensor.matmul(
                        pso[32 * rc:32 * rc + 32, :], w23[:, k, :],
                        h_sb[k][:, 512 * rc:512 * rc + 512],
                        start=(k == 0), stop=(k == MT - 1),
                        tile_position=(0, 32 * rc), skip_group_check=True)
            # g = (out2 + (bep2 + bp3)) + p ; gm = g * z
            nc.vector.scalar_tensor_tensor(
                g_sb[:], pso[:, :], bias_sb[:, 8:9], prep_sb[:],
                op0=ADD, op1=ADD)
            nc.vector.tensor_tensor(gm_sb[:], g_sb[:], zp_sb[:], op=MULT)
            # partition-group sum over the 32 noise dims
            psS = ps_o2.tile([128, 512], F32, tag="po", name="psS")
            nc.tensor.matmul(psS[0:RC, :], sel4_sb[:], gm_sb[:].bitcast(F32R),
                             start=True, stop=True)
            nc.scalar.activation(out_sb[:], psS[0:RC, :], COPY)
            nc.sync.dma_start(out[:], out_sb[:])

    nc.compile()
    return nc


def _prep(x, feature, z, Wep1, bep1, Wep2, bep2, Wp1, bp1, Wp2, bp2, Wp3, bp3):
    """Host-side weight/layout prep shared across cores."""
    c32 = lambda a: np.ascontiguousarray(np.asarray(a, dtype=np.float32))
    xfT = np.ascontiguousarray(
        np.concatenate([x, feature], axis=1).T.astype(NPDT))  # [XF, B]
    # swizzle w1 into SBUF layout [p, m, (k h)]
    w1 = np.ascontiguousarray(
        np.asarray(Wep1, np.float32)[:XF].astype(NPDT)
        .reshape(KT, 128, MT, 128).transpose(1, 2, 0, 3)
        .reshape(128, MT, KT * 128))

    packb = np.zeros((128, PB_COLS), NPDT)
    packb[:, PB_W1Z:PB_W2] = np.tile(np.asarray(Wep1, np.float32)[XF:], (4, 1))
    packb[:, PB_W2:PB_W2A] = (np.asarray(Wep2, np.float32)
                              .reshape(4, 128, ND).transpose(1, 0, 2)
                              .reshape(128, 4 * ND))
    wp2 = np.zeros((PHF, PHF), np.float32)
    wp3 = np.zeros((PHF, ND), np.float32)
    for e in range(ND):
        wp2[5 * e:5 * e + 5, 5 * e:5 * e + 5] = Wp2[e]
        wp3[5 * e:5 * e + 5, e] = np.asarray(Wp3)[e, :, 0]
    packb[:, PB_W2A:PB_W2B] = wp2[0:128]
    packb[0:32, PB_W2B:PB_W3A] = wp2[128:160]
    packb[:, PB_W3A:PB_W3B] = wp3[0:128]
    packb[0:32, PB_W3B:PB_WP1] = wp3[128:160]
    packb[:, PB_WP1:PB_COLS] = (np.asarray(Wp1, np.float32)
                                .transpose(1, 0, 2).reshape(SD, PHF)
                                .reshape(8, 128, PHF).transpose(1, 0, 2)
                                .reshape(128, 8 * PHF))

    bias = np.zeros((128, 9), np.float32)
    bias[:, 0:4] = np.asarray(bep1, np.float32).reshape(4, 128).T
    bp1f = np.asarray(bp1, np.float32).reshape(PHF)
    bp2f = np.asarray(bp2, np.float32).reshape(PHF)
    bias[:, 4] = bp1f[:128]
    bias[:32, 5] = bp1f[128:]
    bias[:, 6] = bp2f[:128]
    bias[:32, 7] = bp2f[128:]
    bias[:, 8] = np.tile(np.asarray(bep2, np.float32)
                         + np.asarray(bp3, np.float32)[:, 0], 4)
    sel4 = np.zeros((128, 4), np.float32)
    sel4[np.arange(128), np.arange(128) // 32] = 1.0
    shared = dict(w1=w1, packb=packb, bias=bias, sel4=sel4)
    in_maps = []
    for c in range(N_CORES):
        sl = slice(c * BL, (c + 1) * BL)
        zTf = np.asarray(z)[sl].transpose(1, 0, 2).reshape(R, ND).T  # [32, R]
        zpk = c32(np.ascontiguousarray(zTf).reshape(ND, RC, 512)
                  .transpose(1, 0, 2).reshape(128, 512))
        m = dict(shared)
        m["xfT"] = np.ascontiguousarray(
            xfT[:, sl].reshape(KT, 128, BL).transpose(1, 0, 2)
            .reshape(128, KT * BL))
        m["zT4"] = np.ascontiguousarray(np.tile(zTf, (4, 1)).astype(NPDT))
        m["zp"] = zpk
        in_maps.append(m)
    return in_maps


def kernel(**inputs):
    if "nc" not in _CACHE:
        _CACHE["nc"] = _build()
    nc = _CACHE["nc"]
    in_maps = _prep(**inputs)
    last_err = None
    for _attempt in range(3):
        try:
            res = run_bass_kernel_spmd(nc, in_maps, list(range(N_CORES)))
            full = np.empty((B, N_Z, 1), np.float32)
            for c in range(N_CORES):
                S = np.asarray(res.results[c]["out"]).reshape(R)
                full[c * BL:(c + 1) * BL, :, 0] = S.reshape(N_Z, BL).T
            return full
        except Exception as e:  # transient device/transfer hiccups
            last_err = e
            time.sleep(5.0 * (_attempt + 1))
    raise last_err



# revision 89
# speedup vs baseline: 1.2757x; 1.2757x over previous
"""Trainium2 Bass kernel for EpiLinear (epinet + prior-ensemble MLP).

Strategy (data-parallel over batch, params replicated, B=2048 -> 256/core):
  - Epinet L1 split: h = relu(A[b] + Bz[b,n]) with A = xf @ Wep1[:2048]
    computed once per batch row (8x fewer FLOPs) and Bz = z @ Wep1[2048:].
    bep1 is folded into Bz via a ones-row appended to z^T.
  - Bz runs in fp8-e4m3 with the DoubleRow perf mode (2 K-groups of 17
    packed in the free dims): half matmul cost; Bz is ~1/8 the magnitude
    of A so the fp8 quantization error is negligible in the output.
  - relu rewrite: h = max(Bz', -A) - (-A). Pool does the max (the PSUM
    read), DVE does the subtract in all-bf16 2x_1p fast mode, Act only
    produces -A (scale=-1 copy). One pass per engine, no bottleneck.
  - Epinet L2 runs TRANSPOSED: out2^T[r, s] with h-tiles as lhsT
    (Ldweights is free), so M=32 wastes no PE columns. The prior net's
    final layer (p = h2 @ wp3 + b) folds into the same PSUM accumulation
    as two extra K-steps (h2 slices as lhsT, ones-row carries the bias).
  - Final reduce: gm = pso * zr (DVE), group-sum over the 32 noise dims
    via free-axis tensor_reduce. Split in halves to overlap the out DMA.
  - PE warmup: dummy matmuls ramp the tensor engine p-state during the
    initial DMA window (cost model: full speed after 3us continuous busy).
  - One SP DMA stream in exact need-order (the global DMA transfer
    resource serializes all queues anyway); PE order A(m0) before the
    prior chain so the x/w1-m0 stream feeds the big GEMM ASAP.
"""

import time

import numpy as np
import ml_dtypes

import concourse.bacc as bacc
import concourse.mybir as mybir
import concourse.tile as tile
from concourse.bass_utils import run_bass_kernel_spmd

F32 = mybir.dt.float32
BF16 = mybir.dt.bfloat16
FP8 = mybir.dt.float8e4
RELU = mybir.ActivationFunctionType.Relu
COPY = mybir.ActivationFunctionType.Copy
ADD = mybir.AluOpType.add
MULT = mybir.AluOpType.mult
MAX = mybir.AluOpType.max
SUB = mybir.AluOpType.subtract
AX_X = mybir.AxisListType.X
DROW = mybir.MatmulPerfMode.DoubleRow

DT = BF16
NPDT = ml_dtypes.bfloat16
NPF8 = ml_dtypes.float8_e4m3

N_CORES = 8
B, N_Z, ND, SD, HD = 2048, 8, 32, 1024, 1024
EH = 512                  # epinet hidden
XF = SD + HD              # 2048 concat(x, feature) features
BL = B // N_CORES         # 256 batch rows per core
R = BL * N_Z              # 2048 epinet rows per core (r = n*BL + b, n-major)
KT = XF // 128            # 16 k-tiles over xf features
MT = EH // 128            # 4 hid tiles of epinet hidden
RT = R // 128             # 16 r-tiles (L2 transposed)
PH = 160                  # 32 ensembles * 5 prior hidden, flattened

# pk2 bf16 block offsets: w2m | wp2a | wp2b | wp3a | wp3b | id | w2n | wp1
PK_W2, PK_W2A, PK_W2B = 0, 128, 288
PK_W3A, PK_W3B, PK_ID = 448, 480, 512
PK_W2N, PK_WP1, PK_COLS = 640, 768, 2048

N_WARM = 5                # warmup matmuls (ap=512) to ramp PE p-state

_CACHE = {}


def _build():
    nc = bacc.Bacc("TRN2", target_bir_lowering=False, debug=False,
                   num_devices=N_CORES)
    f = lambda name, shape, dt: nc.dram_tensor(name, shape, dt, kind="ExternalInput").ap()
    xfT = f("xfT", [128, KT * BL], DT)    # xf.T slice, SBUF-layout swizzled
    w1 = f("w1", [128, MT, KT * 128], DT)  # Wep1[:2048] SBUF-layout swizzled
    zb2 = f("zb2", [17, 2 * R], FP8)      # z^T + ones row, DoubleRow packed
    w1z2 = f("w1z2", [17, 2 * EH], FP8)   # Wep1[2048:] + bep1, DoubleRow
    zr = f("zr", [128, RT * ND], DT)      # z in [r-part, (t, s)] layout
    pk2 = f("pk2", [128, PK_COLS], DT)    # all small bf16 params
    biasv = f("biasv", [128, 4], F32)     # bp1 / bp2 per-partition biases
    out = nc.dram_tensor("out", [128, RT], F32, kind="ExternalOutput").ap()

    with tile.TileContext(nc) as tc:
        with (
            tc.tile_pool(name="const", bufs=1) as cp,
            tc.tile_pool(name="work", bufs=1) as wk,
            tc.tile_pool(name="hp", bufs=4) as hp_pool,
            tc.tile_pool(name="ps_z", bufs=2, space="PSUM") as ps_z,
            tc.tile_pool(name="ps_a", bufs=2, space="PSUM") as ps_a,
            tc.tile_pool(name="ps_s", bufs=1, space="PSUM") as ps_s,
            tc.tile_pool(name="ps_o", bufs=1, space="PSUM") as ps_o,
        ):
            # ---- SBUF tiles -------------------------------------------------
            xfT_sb = cp.tile([128, KT * BL], DT)       # [p, (k b)]
            w1_sb = cp.tile([128, MT * KT * 128], DT)  # [p, (m k h)]
            zb2_sb = cp.tile([17, 2 * R], FP8)
            w1z2_sb = cp.tile([17, 2 * EH], FP8)
            zr_sb = cp.tile([128, RT * ND], DT)
            pk2_sb = cp.tile([128, PK_COLS], DT)
            biasv_sb = cp.tile([128, 4], F32)
            wsc = cp.tile([1, 640], DT)                # warmup scratch

            A_sb = wk.tile([128, MT * BL], DT)         # -A^T, [p, (m b)]
            h_sb = [wk.tile([128, R], DT, name=f"h{m}") for m in range(MT)]
            h1a_sb = wk.tile([128, BL], DT)
            h1b_sb = wk.tile([32, BL], DT)
            h1t_sb = wk.tile([128, 64], DT)    # h1b^T staging (pre-transpose)
            h2a_sb = wk.tile([128, BL], DT)
            h2b_sb = wk.tile([33, BL], DT)             # row 32 = ones (p-bias)
            gm_sb = wk.tile([128, RT * ND], DT)
            S_sb = wk.tile([128, RT], F32)

            x3 = xfT_sb[:].rearrange("p (k b) -> p k b", b=BL)
            w13 = w1_sb[:].rearrange("p (m k h) -> p m k h", m=MT, h=128)
            zb3 = zb2_sb[:].rearrange("p (i r) -> p i r", i=2)
            w1z3 = w1z2_sb[:].rearrange("p (i h) -> p i h", i=2)
            w2m3 = pk2_sb[:, PK_W2:PK_W2A].rearrange("p (m s) -> p m s", s=ND)
            wp2a_v = pk2_sb[:, PK_W2A:PK_W2B]
            wp2b_v = pk2_sb[0:32, PK_W2B:PK_W3A]
            wp3a_v = pk2_sb[:, PK_W3A:PK_W3B]
            wp3b_v = pk2_sb[0:33, PK_W3B:PK_ID]
            id_v = pk2_sb[:, PK_ID:PK_W2N]
            w2n3 = pk2_sb[:, PK_W2N:PK_WP1].rearrange("p (m s) -> p m s", s=ND)
            wp13 = pk2_sb[:, PK_WP1:PK_COLS].rearrange("p (k g) -> p k g", g=PH)

            # ---- memsets (Pool) early: warmup scratch + ones row -----------
            nc.gpsimd.memset(wsc[:], 0.125)
            nc.gpsimd.memset(h2b_sb[32:33, :], 1.0)

            # ---- DMAs: two queues, alternating in global need-order --------
            # (the transfer resource serializes globally; alternating keeps
            # per-queue issue overhead off the critical path)
            nc.sync.dma_start(xfT_sb[:, 0:1024], xfT[:, 0:1024])
            nc.scalar.dma_start(xfT_sb[:, 1024:2048], xfT[:, 1024:2048])
            nc.sync.dma_start(w1_sb[:, 0:1024], w1[:, 0, 0:1024])
            nc.scalar.dma_start(xfT_sb[:, 2048:3072], xfT[:, 2048:3072])
            nc.sync.dma_start(w1_sb[:, 1024:2048], w1[:, 0, 1024:2048])
            nc.scalar.dma_start(xfT_sb[:, 3072:4096], xfT[:, 3072:4096])
            nc.sync.dma_start(zb2_sb[:], zb2[:])
            nc.scalar.dma_start(w1z2_sb[:], w1z2[:])
            nc.sync.dma_start(pk2_sb[:, PK_WP1:PK_WP1 + 640],
                              pk2[:, PK_WP1:PK_WP1 + 640])
            nc.scalar.dma_start(pk2_sb[:, PK_WP1 + 640:PK_COLS],
                                pk2[:, PK_WP1 + 640:PK_COLS])
            nc.sync.dma_start(biasv_sb[:], biasv[:])
            # w1 m1 before pk2s: A(m1) needs it at ~7.5us; the small params
            # (w2/wp2/wp3/id) are only needed by h2/transposes/L2 later.
            # Late w1 halves all on SP: more Act DMAs would backpressure the
            # Act wait-queue (depth 4) and stall Act's activation stream.
            nc.sync.dma_start(w1_sb[:, 2048:3072], w1[:, 1, 0:1024])
            nc.sync.dma_start(w1_sb[:, 3072:4096], w1[:, 1, 1024:2048])
            nc.scalar.dma_start(pk2_sb[:, 0:PK_WP1], pk2[:, 0:PK_WP1])
            for m in (2, 3):
                nc.sync.dma_start(w1_sb[:, 2048 * m:2048 * m + 1024],
                                  w1[:, m, 0:1024])
                nc.sync.dma_start(w1_sb[:, 2048 * m + 1024:2048 * (m + 1)],
                                  w1[:, m, 1024:2048])

            # ---- PE warmup: ramp p-state during DMA window -----------------
            pso = ps_o.tile([128, RT * ND], F32, name="pso")
            for i in range(N_WARM):
                nc.tensor.matmul(pso[:, :], wsc[:, 0:128], wsc[:, 128:640],
                                 start=True, stop=True)

            # One PSUM bank shared by the prior accumulators (two 1KB halves)
            ps_pp = ps_s.tile([128, 2 * BL], F32, name="ps_pp")
            pA, pB = ps_pp[:, 0:BL], ps_pp[:, BL:2 * BL]

            # ---- main loop: A(m) + Bz(m) -> h(m); prior chain after m=0 ----
            def emit_bz(m, half):
                psz = ps_z.tile([128, 1024], F32, tag="pz",
                                name=f"pz{m}_{half}")
                for q in range(2):
                    c0 = 1024 * half + 512 * q
                    nc.tensor.matmul(
                        psz[:, 512 * q:512 * (q + 1)],
                        w1z3[:, :, 128 * m:128 * (m + 1)],
                        zb3[:, :, c0:c0 + 512],
                        start=True, stop=True, perf_mode=DROW)
                return psz

            for m in range(MT):
                pzh = {}
                psA = ps_a.tile([128, BL], F32, tag="pa", name=f"psA{m}")
                for k in range(KT):
                    if m == 0 and k == 12:
                        # fill A0's xfT-tail DMA stall with the Bz matmuls
                        # (zb2/w1z2 land ~1us before the last xfT chunk)
                        pzh[0] = emit_bz(0, 0)
                        pzh[1] = emit_bz(0, 1)
                    nc.tensor.matmul(psA, w13[:, m, k, :], x3[:, k, :],
                                     start=(k == 0), stop=(k == KT - 1))
                # -A in bf16 (Act), consumed by Pool max and DVE subtract
                nc.scalar.activation(A_sb[:, BL * m:BL * (m + 1)], psA,
                                     COPY, scale=-1.0)
                Av2 = A_sb[:, BL * m:BL * (m + 1)].unsqueeze(1).broadcast_to(
                    (128, 2, BL))
                # h_sb stores t = max(Bz', -A); the "- (-A)" completes inside
                # the L2 matmul (extra K-steps with -W2), so each chunk needs
                # exactly ONE psum-side pass:
                #   alpha: DVE max(psz, -A) -> t  (GPSIMD cannot read PSUM)
                #   beta:  Act copy psz -> sbuf, Pool max -> t
                # alpha chunks: one DVE op max(psz, -A) (only one PSUM input
                # is allowed per DVE instruction, so -A comes from SBUF)
                if m < MT - 1:
                    # halves: h0 via Act-copy + DVE-max, h1 via DVE-max
                    for half in range(2):
                        psz = pzh.get(half)
                        if psz is None:
                            psz = emit_bz(m, half)
                        hv = h_sb[m][:, 1024 * half:1024 * (half + 1)] \
                            .rearrange("p (a b) -> p a b", a=4)
                        if half == 0:
                            # Act copies psum->sbuf; DVE maxes in 2x_1p mode
                            # (all-bf16 sbuf operands)
                            Av4 = A_sb[:, BL * m:BL * (m + 1)].unsqueeze(1) \
                                .broadcast_to((128, 4, BL))
                            zc = hp_pool.tile([128, 1024], DT, tag="hp")
                            nc.scalar.activation(zc[:], psz[:, :], COPY)
                            nc.vector.tensor_tensor(
                                hv, zc[:].rearrange("p (a b) -> p a b", a=4),
                                Av4, op=MAX)
                        else:
                            Av4 = A_sb[:, BL * m:BL * (m + 1)].unsqueeze(1) \
                                .broadcast_to((128, 4, BL))
                            nc.vector.tensor_tensor(
                                hv,
                                psz[:, :].rearrange("p (a b) -> p a b", a=4),
                                Av4, op=MAX)
                else:
                    # m3: quarter granularity for the shortest tail chain
                    qorder = (3, 2, 0, 1)
                    act_beta = {2, 1}
                    pz = {}
                    for q in qorder:
                        half = q // 2
                        if half not in pz:
                            pz[half] = ps_z.tile([128, 1024], F32, tag="pz",
                                                 name=f"pz{m}_{half}")
                        c0 = 512 * q
                        nc.tensor.matmul(
                            pz[half][:, 512 * (q % 2):512 * (q % 2) + 512],
                            w1z3[:, :, 128 * m:128 * (m + 1)],
                            zb3[:, :, c0:c0 + 512],
                            start=True, stop=True, perf_mode=DROW)
                    for q in qorder:
                        psz_v = pz[q // 2][:, 512 * (q % 2):
                                           512 * (q % 2) + 512]
                        hv = h_sb[m][:, 512 * q:512 * (q + 1)].rearrange(
                            "p (a b) -> p a b", a=2)
                        if q in act_beta:
                            zc = hp_pool.tile([128, 512], DT, tag="hp")
                            nc.scalar.activation(zc[:], psz_v, COPY)
                            nc.vector.tensor_tensor(
                                hv,
                                zc[:].rearrange("p (a b) -> p a b", a=2),
                                Av2, op=MAX)
                        else:
                            nc.vector.tensor_tensor(
                                hv,
                                psz_v.rearrange("p (a b) -> p a b", a=2),
                                Av2, op=MAX)
                if m == 0:
                    # ---- prior ensemble: h1 = relu(x @ wp1 + bp1) ----------
                    # (relu+bias on DVE: Act is busy with -A copies here)
                    for k in range(8):
                        nc.tensor.matmul(pA, wp13[:, k, 0:128], x3[:, k, :],
                                         start=(k == 0), stop=(k == 7),
                                         skip_group_check=True)
                    # last 32 prior units computed TRANSPOSED (ap=32 not 256)
                    for bt in range(2):
                        for k in range(8):
                            nc.tensor.matmul(
                                pB[:, 32 * bt:32 * bt + 32],
                                x3[:, k, 128 * bt:128 * bt + 128],
                                wp13[:, k, 128:PH], start=(k == 0),
                                stop=(k == 7), skip_group_check=True)
                    nc.vector.tensor_scalar(h1a_sb[:], pA,
                                            biasv_sb[:, 0:1], 0.0,
                                            op0=ADD, op1=MAX)
                    nc.scalar.activation(h1t_sb[:], pB[:, 0:64], COPY)
                if m == 1:
                    # transpose h1b^T back to [32, 256] (PE, all-bf16 via a
                    # bf16 view of the pB bank), then relu+bias on DVE
                    tps = pB.bitcast(DT)   # [128, 512] bf16 view
                    for bt in range(2):
                        nc.tensor.transpose(
                            tps[0:32, 128 * bt:128 * bt + 128],
                            h1t_sb[:, 32 * bt:32 * bt + 32], id_v[:, 0:128])
                    nc.vector.tensor_scalar(h1b_sb[:], tps[0:32, 0:256],
                                            biasv_sb[0:32, 1:2], 0.0,
                                            op0=ADD, op1=MAX)
                    # h2 = relu(h1 @ wp2 + bp2)   (block-diag dense)
                    nc.tensor.matmul(pA, wp2a_v[:, 0:128], h1a_sb[:],
                                     start=True, stop=False,
                                     skip_group_check=True)
                    nc.tensor.matmul(pA, wp2b_v[:, 0:128], h1b_sb[:],
                                     start=False, stop=True,
                                     skip_group_check=True)
                    nc.tensor.matmul(pB[0:32, :], wp2a_v[:, 128:PH],
                                     h1a_sb[:], start=True, stop=False,
                                     skip_group_check=True)
                    nc.tensor.matmul(pB[0:32, :], wp2b_v[:, 128:PH],
                                     h1b_sb[:], start=False, stop=True,
                                     skip_group_check=True)
                    nc.scalar.activation(h2a_sb[:], pA, RELU,
                                         bias=biasv_sb[:, 2:3])
                    nc.scalar.activation(h2b_sb[0:32, :], pB[0:32, :], RELU,
                                         bias=biasv_sb[0:32, 3:4])
                if m == 2:
                    # zr is only needed by the finale; queue it late, off-SP
                    nc.scalar.dma_start(zr_sb[:], zr[:])

            # ---- epinet L2 transposed + prior-p fold: pso[r, (t,s)] --------
            # then finale per quarter: gm = pso * zr ; S = group-sum over s
            # piece order matches m3's h completion order: alpha quarters
            # (q3, q0) finish first on DVE, then the Act-copy betas (q2, q1),
            # with q1 split into eighths for the shortest final chain
            for t0, t1 in ((12, 16), (0, 4), (8, 12), (4, 8)):
                for t in range(t0, t1):
                    bh = 128 * (t % 2)
                    o = pso[:, ND * t:ND * (t + 1)]
                    for m in range(MT):
                        nc.tensor.matmul(o, h_sb[m][:, 128 * t:128 * (t + 1)],
                                         w2m3[:, m, :], start=(m == 0),
                                         stop=False)
                        # completes h = t - (-A): subtract (-A)'s contribution
                        nc.tensor.matmul(o, A_sb[:, BL * m + bh:
                                                  BL * m + bh + 128],
                                         w2n3[:, m, :], start=False,
                                         stop=False)
                    nc.tensor.matmul(o, h2a_sb[:, bh:bh + 128], wp3a_v[:],
                                     start=False, stop=False)
                    nc.tensor.matmul(o, h2b_sb[:, bh:bh + 128], wp3b_v[:],
                                     start=False, stop=True)
                c0, c1 = ND * t0, ND * t1
                if t0 in (12, 8, 4):  # direct on DVE (no Act-copy hop)
                    nc.vector.tensor_tensor(gm_sb[:, c0:c1], pso[:, c0:c1],
                                            zr_sb[:, c0:c1], op=MULT)
                else:         # Act moves psum->sbuf; DVE multiplies (2x_1p)
                    gc = hp_pool.tile([128, c1 - c0], DT, tag="gc")
                    nc.scalar.activation(gc[:], pso[:, c0:c1], COPY)
                    nc.vector.tensor_tensor(gm_sb[:, c0:c1], gc[:],
                                            zr_sb[:, c0:c1], op=MULT)
                nc.vector.tensor_reduce(
                    S_sb[:, t0:t1],
                    gm_sb[:, c0:c1].rearrange("p (t s) -> p t s", s=ND),
                    axis=AX_X, op=ADD)
                nc.sync.dma_start(out[:, t0:t1], S_sb[:, t0:t1])

    nc.compile()
    return nc


def _prep(x, feature, z, Wep1, bep1, Wep2, bep2, Wp1, bp1, Wp2, bp2, Wp3, bp3):
    """Host-side weight/layout prep shared across cores."""
    xfT = np.ascontiguousarray(
        np.concatenate([x, feature], axis=1).T.astype(NPDT))  # [XF, B]
    w1 = np.ascontiguousarray(
        np.asarray(Wep1, np.float32)[:XF].astype(NPDT)
        .reshape(KT, 128, MT, 128).transpose(1, 2, 0, 3)
        .reshape(128, MT, KT * 128))

    # DoubleRow-packed fp8 z-weights: rows 0..31 = Wep1[2048:], row 32 =
    # bep1, row 33 = 0; split into 2 K-groups of 17.
    w1zf = np.zeros((34, EH), np.float32)
    w1zf[0:32] = np.asarray(Wep1, np.float32)[XF:]
    w1zf[32] = np.asarray(bep1, np.float32)
    w1z2 = np.ascontiguousarray(
        w1zf.reshape(2, 17, EH).transpose(1, 0, 2)
        .reshape(17, 2 * EH).astype(NPF8))

    pk2 = np.zeros((128, PK_COLS), NPDT)
    pk2[:, PK_W2:PK_W2A] = (np.asarray(Wep2, np.float32)
                            .reshape(MT, 128, ND).transpose(1, 0, 2)
                            .reshape(128, MT * ND))
    wp2f = np.zeros((PH, PH), np.float32)
    wp3g = np.zeros((PH, ND), np.float32)
    for e in range(ND):
        wp2f[5 * e:5 * e + 5, 5 * e:5 * e + 5] = Wp2[e]
        wp3g[5 * e:5 * e + 5, e] = np.asarray(Wp3)[e, :, 0]
    pk2[:, PK_W2A:PK_W2B] = wp2f[0:128]
    pk2[0:32, PK_W2B:PK_W3A] = wp2f[128:PH]
    pk2[:, PK_W3A:PK_W3B] = wp3g[0:128]
    pk2[0:32, PK_W3B:PK_ID] = wp3g[128:PH]
    pk2[32, PK_W3B:PK_ID] = (np.asarray(bep2, np.float32)
                             + np.asarray(bp3, np.float32)[:, 0])
    pk2[:, PK_ID:PK_W2N] = np.eye(128, dtype=np.float32)
    pk2[:, PK_W2N:PK_WP1] = -np.asarray(pk2[:, PK_W2:PK_W2A], np.float32)
    pk2[:, PK_WP1:PK_COLS] = (np.asarray(Wp1, np.float32)
                              .transpose(1, 0, 2).reshape(SD, PH)
                              .reshape(8, 128, PH).transpose(1, 0, 2)
                              .reshape(128, 8 * PH))

    biasv = np.zeros((128, 4), np.float32)
    bp1f = np.asarray(bp1, np.float32).reshape(PH)
    bp2f = np.asarray(bp2, np.float32).reshape(PH)
    biasv[:, 0] = bp1f[:128]
    biasv[0:32, 1] = bp1f[128:]
    biasv[:, 2] = bp2f[:128]
    biasv[0:32, 3] = bp2f[128:]

    shared = dict(w1=w1, pk2=pk2, biasv=biasv, w1z2=w1z2)
    in_maps = []
    for c in range(N_CORES):
        sl = slice(c * BL, (c + 1) * BL)
        zT = np.asarray(z, np.float32)[sl].transpose(1, 0, 2).reshape(R, ND).T
        zp = np.zeros((34, R), np.float32)
        zp[0:32] = zT
        zp[32] = 1.0
        m = dict(shared)
        m["xfT"] = np.ascontiguousarray(
            xfT[:, sl].reshape(KT, 128, BL).transpose(1, 0, 2)
            .reshape(128, KT * BL))
        m["zb2"] = np.ascontiguousarray(
            zp.reshape(2, 17, R).transpose(1, 0, 2)
            .reshape(17, 2 * R).astype(NPF8))
        m["zr"] = np.ascontiguousarray(
            zT.reshape(ND, RT, 128).transpose(2, 1, 0)
            .reshape(128, RT * ND).astype(NPDT))
        in_maps.append(m)
    return in_maps


def kernel(**inputs):
    if "nc" not in _CACHE:
        _CACHE["nc"] = _build()
    nc = _CACHE["nc"]
    in_maps = _prep(**inputs)
    last_err = None
    for _attempt in range(3):
        try:
            res = run_bass_kernel_spmd(nc, in_maps, list(range(N_CORES)))
            full = np.empty((B, N_Z, 1), np.float32)
            for c in range(N_CORES):
                S = np.asarray(res.results[c]["out"])  # [128, RT]
                # r = t*128 + p and r = n*BL + b -> S.T.flat[r] is [n, b]
                full[c * BL:(c + 1) * BL, :, 0] = S.T.reshape(N_Z, BL).T
            return full
        except Exception as e:  # transient device/transfer hiccups
            last_err = e
            time.sleep(5.0 * (_attempt + 1))
    raise last_err
